# revision 41
# baseline (speedup 1.0000x reference)
"""GraphSAGE 2-layer GNN + MLP head on 8 Trainium2 NeuronCores (v11).

Strategy (dst-sharded, dense-adjacency scatter, fp8 DoubleRow):
  - Destination nodes sharded across 8 cores; node index space padded to
    1280 slots/core (10240 global slots = 80 full 128-chunks) so every
    matmul chunk is full and fp8 DoubleRow pairs align.
  - The aggregate-side weight is folded into the scattered values:
    yw = (relu(X W) @ aggw_b), so  h = aggw_t^T x + yw^T A_mean  and the
    scatter matmuls accumulate the h-preactivation directly in PSUM.
    The x-side matmuls open the PSUM groups early (during idle windows).
    A_mean[src,dst] = edge_count/deg(dst) in fp8 e4m3 (mean folded in
    host-side), resident in SBUF and reused by both layers.
    Scatter matmuls run in fp8 DoubleRow (K=256 per instruction).
  - A is stored partition-major in DRAM ([128, 80, 1250]); slab
    dma_starts with 10KB/partition descriptors saturate HBM; y0w and
    the layer-0 scatter are emitted interleaved per slab.
  - Matmul copy/relu work is batched 4 chunks per PSUM bank (512B
    quarter-bank outputs, one wide vector op per batch).
  - Inter-layer AllGather of fp8 y1w split in two (6+4 chunks): the
    second collective overlaps the first half's scatter matmuls; a
    warm-up collective pre-pays the CC engine's cold-start.
  - Row L2-norm: ones-matmul partition reduction, scalar Sqrt, DVE fast
    reciprocal - all partition-parallel.
  - log_softmax: post_mp collapsed host-side to W12 = mp_w1 @ mp_w2; the
    single matmul emits node-major logits; exp/ln batched.
"""

import numpy as np
import ml_dtypes

import concourse.bacc as bacc
import concourse.mybir as mybir
from concourse import tile
from concourse.bass_utils import run_bass_kernel_spmd

N_NODES = 10000
N_CORES = 8
SHARD = N_NODES // N_CORES   # 1250 real dst nodes per core
P = 128
JC = 10                      # local 128-chunks per core (1280 slots)
SLOTS = JC * P               # 1280 padded slots per core
G = N_CORES * SLOTS          # 10240 padded global slots
KC = G // P                  # 80 src chunks
KP = KC // 2                 # 40 DoubleRow pairs
F = 128
FOUT = 64
NCHUNKS = [(0, 512), (512, 512), (1024, SHARD - 1024)]
JA = 6                       # allgather half A: local chunks 0..5
ASLAB = 8                    # a8 chunks per dma slab

FP8 = mybir.dt.float8e4
BF16 = mybir.dt.bfloat16
F32 = mybir.dt.float32
DR = mybir.MatmulPerfMode.DoubleRow
AF = mybir.ActivationFunctionType

NP_FP8 = ml_dtypes.float8_e4m3
NP_BF16 = ml_dtypes.bfloat16


def _jc(j):
    """real node count in local chunk j (last chunk is partial: 98)."""
    return min(P, SHARD - j * P)


def build():
    nc = bacc.Bacc("TRN2", target_bir_lowering=False, debug=False,
                   num_devices=N_CORES)

    # ---- external I/O ----
    xt_d = nc.declare_dram_parameter("xt", [P, G], FP8, isOutput=False)
    xtsh_d = nc.declare_dram_parameter("xt_sh", [P, SLOTS], BF16, isOutput=False)
    a8_d = nc.declare_dram_parameter("a8", [P, KC, SHARD], FP8, isOutput=False)
    lin_w0_d = nc.declare_dram_parameter("lin_w0", [F, F], FP8, isOutput=False)
    lin_w1_d = nc.declare_dram_parameter("lin_w1", [F, F], BF16, isOutput=False)
    agg_w0_d = nc.declare_dram_parameter("agg_w0", [2 * F, F], BF16, isOutput=False)
    agg_w1_d = nc.declare_dram_parameter("agg_w1", [2 * F, F], BF16, isOutput=False)
    mp_w12_d = nc.declare_dram_parameter("mp_w12", [F, FOUT], BF16,
                                         isOutput=False)
    out_d = nc.declare_dram_parameter("out", [SHARD, FOUT], F32, isOutput=True)

    # internal DRAM for the collectives
    warm_in_d = nc.dram_tensor("warm_in_d", [1, 128], FP8)
    warm_out_d = nc.dram_tensor("warm_out_d", [N_CORES, 1, 128], FP8,
                                addr_space="Shared")
    y1sh_a_d = nc.dram_tensor("y1sh_a_d", [P, JA * F], FP8)
    y1sh_b_d = nc.dram_tensor("y1sh_b_d", [P, (JC - JA) * F], FP8)
    y1all_a_d = nc.dram_tensor("y1all_a_d", [N_CORES, P, JA * F], FP8,
                               addr_space="Shared")
    y1all_b_d = nc.dram_tensor("y1all_b_d", [N_CORES, P, (JC - JA) * F], FP8,
                               addr_space="Shared")

    with tile.TileContext(nc) as tc:
        with (
            tc.tile_pool(name="persist", bufs=1) as pp,
            tc.tile_pool(name="work", bufs=2) as wp,
            tc.tile_pool(name="ps_s", bufs=1, space="PSUM") as ps_s,
            tc.tile_pool(name="ps_b", bufs=2, space="PSUM") as ps_b,
            tc.tile_pool(name="ps_y", bufs=2, space="PSUM") as ps_y,
        ):
            # ---- persistent SBUF ----
            a_sb = pp.tile([P, KC, SHARD], FP8)
            xt_sb = pp.tile([P, G], FP8)
            xtsh_sb = pp.tile([P, SLOTS], BF16)
            y0fm = pp.tile([P, G], BF16)
            y1fm = pp.tile([P, SHARD], BF16)
            y_sb = pp.tile([P, KC, F], FP8)
            y1loc = pp.tile([P, JC, F], FP8)
            x1T = pp.tile([P, SHARD], BF16)
            x2T = pp.tile([P, SHARD], BF16)
            z2sb = pp.tile([P, JC, FOUT], F32)
            zc = pp.tile([P, JC, FOUT], F32)
            expall = pp.tile([P, JC, FOUT], F32)
            outsb = pp.tile([P, JC, FOUT], F32)
            rmax = pp.tile([P, JC], F32)
            negmax = pp.tile([P, JC], F32)
            sumexp = pp.tile([P, JC], F32)
            lnsum = pp.tile([P, JC], F32)
            neglns = pp.tile([P, JC], F32)
            lin_w0_sb = pp.tile([F, F], FP8)
            lin_w1_sb = pp.tile([F, F], BF16)
            aggw0t_sb = pp.tile([F, F], BF16)
            aggw0b_sb = pp.tile([F, F], BF16)
            aggw1t_sb = pp.tile([F, F], BF16)
            aggw1b_sb = pp.tile([F, F], BF16)
            mp_w12_sb = pp.tile([F, FOUT], BF16)
            ones_mat = pp.tile([P, P], BF16)
            eps_sb = pp.tile([P, 1], F32)

            # warm-up collective: pre-pays CC cold start + rendezvous
            nc.gpsimd.collective_compute(
                "AllGather", mybir.AluOpType.bypass,
                replica_groups=[list(range(N_CORES))],
                ins=[warm_in_d[:]], outs=[warm_out_d[:]],
            )
            # ---- front loads: y0 + layer-0 deps first (DMA queues are FIFO,
            # ---- so xt must fully precede the big a8 stream) ----
            nc.sync.dma_start(lin_w0_sb[:], lin_w0_d[:])
            XH = G // 2
            nc.sync.dma_start(xt_sb[:, 0:XH], xt_d[:, 0:XH])
            nc.sync.dma_start(xt_sb[:, XH:G], xt_d[:, XH:G])
            nc.sync.dma_start(xtsh_sb[:], xtsh_d[:])
            nc.sync.dma_start(aggw0t_sb[:], agg_w0_d[0:F, :])
            nc.sync.dma_start(aggw0b_sb[:], agg_w0_d[F:2 * F, :])
            for s in range(8):
                nc.sync.dma_start(a_sb[:, s * ASLAB:(s + 1) * ASLAB, :],
                                  a8_d[:, s * ASLAB:(s + 1) * ASLAB, :])
            for s in range(4):
                k0 = 64 + s * 4
                nc.sync.dma_start(a_sb[:, k0:k0 + 4, :],
                                  a8_d[:, k0:k0 + 4, :])
            nc.sync.dma_start(lin_w1_sb[:], lin_w1_d[:])
            nc.sync.dma_start(aggw1t_sb[:], agg_w1_d[0:F, :])
            nc.sync.dma_start(aggw1b_sb[:], agg_w1_d[F:2 * F, :])
            nc.sync.dma_start(mp_w12_sb[:], mp_w12_d[:])
            nc.gpsimd.memset(ones_mat[:], 1.0)
            nc.gpsimd.memset(eps_sb[:], 1e-24)
            nc.gpsimd.memset(y1loc[:, JC - 1, :], 0.0)
            nc.gpsimd.memset(rmax[:], 0.0)
            nc.gpsimd.memset(zc[:, :, :], 0.0)

            def yw_batch(dst_tile, fm_tile, aggwb, chunks4, mcols=None):
                """yw = fm @ aggw_b for 4 node chunks into quarter-bank psum
                slots + one wide fp8 quantize copy."""
                ps = ps_y.tile([P, 512], F32, tag="ps_y", name="ps_ywb")
                for q, k in enumerate(chunks4):
                    mc = P if mcols is None else mcols[q]
                    nc.tensor.matmul(ps[0:mc, q * F:(q + 1) * F],
                                     fm_tile[:, k * P:k * P + mc], aggwb[:],
                                     start=True, stop=True,
                                     skip_group_check=True)
                if mcols is None:
                    nc.vector.tensor_scalar_mul(
                        dst_tile[:, chunks4[0]:chunks4[0] + 4, :], ps[:], 1.0)
                return ps

            def scatter(ps_list, kps, last):
                """fp8 DoubleRow scatter matmuls accumulating into the three
                h-preactivation banks (groups opened by the x-side matmuls);
                kp-outer for DMA pacing."""
                for kp in kps:
                    for i, (n0, ns) in enumerate(NCHUNKS):
                        nc.tensor.matmul(
                            ps_list[i][:, 0:ns],
                            y_sb[:, 2 * kp:2 * kp + 2, :],
                            a_sb[:, 2 * kp:2 * kp + 2, n0:n0 + ns],
                            start=False, stop=(kp == last),
                            perf_mode=DR,
                        )

            def scatter_tail(ps_list, kps):
                """final scatter group, n-chunk outer: bank i stops as soon
                as its own pairs are done, so sage_update pipelines with the
                remaining banks' matmuls."""
                for i, (n0, ns) in enumerate(NCHUNKS):
                    for kp in kps:
                        nc.tensor.matmul(
                            ps_list[i][:, 0:ns],
                            y_sb[:, 2 * kp:2 * kp + 2, :],
                            a_sb[:, 2 * kp:2 * kp + 2, n0:n0 + ns],
                            start=False, stop=(kp == kps[-1]),
                            perf_mode=DR,
                        )

            def xside(ps_list, aggwt_sb, xrhs):
                """open the h psum groups with the x-side contribution."""
                for i, (n0, ns) in enumerate(NCHUNKS):
                    nc.tensor.matmul(ps_list[i][:, 0:ns], aggwt_sb[:],
                                     xrhs[:, n0:n0 + ns],
                                     start=True, stop=False)

            def sage_update(ps_list, xout):
                """relu + L2 row norm straight from the h psum banks."""
                for i, (n0, ns) in enumerate(NCHUNKS):
                    ps = ps_list[i]
                    hT = wp.tile([P, 512], F32, tag="hT")
                    nc.vector.tensor_scalar_max(hT[:, 0:ns], ps[:, 0:ns], 0.0)
                    h2 = wp.tile([P, 512], BF16, tag="h2")
                    nc.vector.tensor_tensor(h2[:, 0:ns], hT[:, 0:ns],
                                            hT[:, 0:ns], mybir.AluOpType.mult)
                    pb = ps_b.tile([P, 512], F32, tag="pb")
                    nc.tensor.matmul(pb[:, 0:ns], ones_mat[:], h2[:, 0:ns],
                                     start=True, stop=True)
                    nrm = wp.tile([P, 512], F32, tag="nrm")
                    nc.scalar.activation(nrm[:, 0:ns], pb[:, 0:ns], AF.Sqrt,
                                         bias=eps_sb[:])
                    rinv = wp.tile([P, 512], F32, tag="rinv")
                    nc.vector.reciprocal_approx_fast(rinv[:, 0:ns],
                                                     nrm[:, 0:ns])
                    nc.vector.tensor_tensor(xout[:, n0:n0 + ns], hT[:, 0:ns],
                                            rinv[:, 0:ns],
                                            mybir.AluOpType.mult)

            # ---- layer 0: x-side opens the psum groups; y0w = relu(x@w0)
            # ---- @aggw0b interleaved with the a8-paced scatter ----
            ps_l0 = [ps_s.tile([P, 512], F32, tag=f"s{i}", name=f"ps_l0_{i}")
                     for i in range(3)]
            xside(ps_l0, aggw0t_sb, xtsh_sb)
            SLAB_KPS = ([list(range(4 * s, 4 * s + 4)) for s in range(8)] +
                        [list(range(32 + 2 * s, 34 + 2 * s)) for s in range(4)])
            for s, kps in enumerate(SLAB_KPS):
                for kp in kps:
                    k0 = 2 * kp
                    if k0 % 4 == 0:
                        # y0 feature-major for 4 chunks (one 512-col matmul)
                        psf = ps_y.tile([P, 512], F32, tag="ps_y",
                                        name="ps_y0fm")
                        nc.tensor.matmul(psf[:], lin_w0_sb[:],
                                         xt_sb[:, k0 * P:(k0 + 4) * P],
                                         start=True, stop=True)
                        nc.vector.tensor_scalar_max(
                            y0fm[:, k0 * P:(k0 + 4) * P], psf[:], 0.0)
                        yw_batch(y_sb, y0fm, aggw0b_sb,
                                 list(range(k0, k0 + 4)))
                if s < len(SLAB_KPS) - 1:
                    scatter(ps_l0, kps, -1)
                else:
                    scatter_tail(ps_l0, kps)
            sage_update(ps_l0, x1T)

            # ---- y1w = (relu(x1 @ lin_w1)) @ aggw1b; split AllGather ----
            for i, (n0, ns) in enumerate(NCHUNKS):
                psf = ps_y.tile([P, 512], F32, tag="ps_y", name="ps_y1fm")
                nc.tensor.matmul(psf[:, 0:ns], lin_w1_sb[:],
                                 x1T[:, n0:n0 + ns], start=True, stop=True)
                nc.vector.tensor_scalar_max(y1fm[:, n0:n0 + ns],
                                            psf[:, 0:ns], 0.0)
            yw_batch(y1loc, y1fm, aggw1b_sb, [0, 1, 2, 3])
            ps45 = yw_batch(y1loc, y1fm, aggw1b_sb, [4, 5], mcols=[P, P])
            nc.vector.tensor_scalar_mul(y1loc[:, 4:6, :], ps45[:, 0:2 * F], 1.0)

            nc.sync.dma_start(y1sh_a_d[:], y1loc[:, 0:JA, :])
            nc.gpsimd.collective_compute(
                "AllGather", mybir.AluOpType.bypass,
                replica_groups=[list(range(N_CORES))],
                ins=[y1sh_a_d[:]], outs=[y1all_a_d[:]],
            )
            # x-side of layer 1 fills the collective window
            ps_l1 = [ps_s.tile([P, 512], F32, tag=f"s{i}", name=f"ps_l1_{i}")
                     for i in range(3)]
            xside(ps_l1, aggw1t_sb, x1T)

            jc9 = _jc(JC - 1)
            ps69 = yw_batch(y1loc, y1fm, aggw1b_sb, [6, 7, 8, 9],
                            mcols=[P, P, P, jc9])
            nc.vector.tensor_scalar_mul(y1loc[:, 6:9, :], ps69[:, 0:3 * F], 1.0)
            nc.vector.tensor_scalar_mul(y1loc[0:jc9, 9, :],
                                        ps69[0:jc9, 3 * F:4 * F], 1.0)

            nc.sync.dma_start(y1sh_b_d[:], y1loc[:, JA:JC, :])
            nc.gpsimd.collective_compute(
                "AllGather", mybir.AluOpType.bypass,
                replica_groups=[list(range(N_CORES))],
                ins=[y1sh_b_d[:]], outs=[y1all_b_d[:]],
            )

            # reload gathered y1w into y_sb (chunk k = c*JC + j)
            ysb_v = y_sb[:, :, :].rearrange("p (c j) f -> p c (j f)", c=N_CORES)
            nc.sync.dma_start(ysb_v[:, :, 0:2 * F],
                              y1all_a_d[:, :, 0:2 * F].transpose([1, 0, 2]))
            nc.sync.dma_start(ysb_v[:, :, 2 * F:JA * F],
                              y1all_a_d[:, :, 2 * F:JA * F].transpose([1, 0, 2]))
            nc.sync.dma_start(ysb_v[:, :, JA * F:JC * F],
                              y1all_b_d[:].transpose([1, 0, 2]))

            # ---- layer 1: scatter half A first (overlaps AllGather B) ----
            kps_a1 = [c * (JC // 2) for c in range(N_CORES)]
            kps_a2 = [c * (JC // 2) + q for c in range(N_CORES) for q in (1, 2)]
            kps_b = [c * (JC // 2) + q for c in range(N_CORES)
                     for q in range(JA // 2, JC // 2)]
            scatter(ps_l1, kps_a1, -1)
            scatter(ps_l1, kps_a2, -1)
            scatter_tail(ps_l1, kps_b)
            # preload Exp's act table before the softmax needs it
            tblscr = wp.tile([P, 1], F32, tag="tblscr")
            nc.scalar.activation(tblscr[:], eps_sb[:], AF.Exp)
            sage_update(ps_l1, x2T)

            # ---- post_mp: z2 = x2 @ (mp_w1 @ mp_w2), node-major logits;
            # ---- max/shift/exp per batch so softmax starts early ----
            for g in range(2):  # z2 in two batches of <=8 chunks per bank
                j0, j1 = (0, 8) if g == 0 else (8, JC)
                pz = ps_y.tile([P, 512], F32, tag="ps_y", name=f"ps_z{g}")
                for q, j in enumerate(range(j0, j1)):
                    jc = _jc(j)
                    nc.tensor.matmul(pz[0:jc, q * FOUT:(q + 1) * FOUT],
                                     x2T[:, j * P:j * P + jc], mp_w12_sb[:],
                                     start=True, stop=True,
                                     skip_group_check=True)
                nb = (j1 - j0) * FOUT
                nc.scalar.activation(
                    z2sb[:, j0:j1, :].rearrange("p j f -> p (j f)"),
                    pz[:, 0:nb], AF.Copy)
                nc.vector.tensor_reduce(rmax[:, j0:j1], z2sb[:, j0:j1, :],
                                        mybir.AxisListType.X,
                                        mybir.AluOpType.max)
                nc.vector.tensor_scalar_mul(negmax[:, j0:j1], rmax[:, j0:j1],
                                            -1.0)
                for j in range(j0, j1):
                    jc = _jc(j)
                    nc.vector.tensor_scalar_add(zc[0:jc, j, :],
                                                z2sb[0:jc, j, :],
                                                negmax[0:jc, j:j + 1])
                nc.scalar.activation(expall[:, j0:j1, :], zc[:, j0:j1, :],
                                     AF.Exp)
                nc.vector.tensor_reduce(sumexp[:, j0:j1],
                                        expall[:, j0:j1, :],
                                        mybir.AxisListType.X,
                                        mybir.AluOpType.add)

            # ---- log_softmax over classes, batched ----
            nc.scalar.activation(lnsum[:], sumexp[:], AF.Ln)
            nc.vector.tensor_scalar_mul(neglns[:], lnsum[:], -1.0)
            for j in range(JC):
                jc = _jc(j)
                nc.vector.tensor_scalar_add(outsb[0:jc, j, :], zc[0:jc, j, :],
                                            neglns[0:jc, j:j + 1])
            nfull = (JC - 1) * P  # 1152 nodes in full chunks
            nc.sync.dma_start(
                out_d[0:nfull, :].rearrange("(j p) f -> p j f", p=P),
                outsb[:, 0:JC - 1, :])
            nc.sync.dma_start(out_d[nfull:SHARD, :],
                              outsb[0:_jc(JC - 1), JC - 1, :])

    nc.compile()
    return nc


_NC = None


def _get_nc():
    global _NC
    if _NC is None:
        _NC = build()
    return _NC


def make_in_maps(inputs):
    x = np.asarray(inputs["x"], dtype=np.float32)
    ei = np.asarray(inputs["edge_index"])
    src = ei[0].astype(np.int64)
    dst = ei[1].astype(np.int64)

    cnt = np.bincount(dst, minlength=N_NODES).astype(np.float32)
    inv = (1.0 / np.maximum(cnt, 1.0)).astype(np.float32)

    # dense scatter-mean matrix: edge_count/deg(dst), padded src slots,
    # partition-major per core
    srcp = (src // SHARD) * SLOTS + (src % SHARD)
    flat = srcp * N_NODES + dst
    counts = np.bincount(flat, minlength=G * N_NODES)
    A = counts.reshape(G, N_NODES).astype(np.float32)
    del counts
    A *= inv[None, :]
    A8 = A.astype(NP_FP8).reshape(KC, P, N_NODES).transpose(1, 0, 2)
    del A

    # padded transposed features [128, 10240]
    xp = np.zeros((G, F), np.float32)
    for c in range(N_CORES):
        xp[c * SLOTS:c * SLOTS + SHARD] = x[c * SHARD:(c + 1) * SHARD]
    xt8 = np.ascontiguousarray(xp.T).astype(NP_FP8)
    xt16 = np.ascontiguousarray(xp.T).astype(NP_BF16)

    def w(name, dt=NP_BF16):
        return np.ascontiguousarray(
            np.asarray(inputs[name], np.float32)).astype(dt)

    w12 = np.asarray(inputs["mp_w1"], np.float32) @ np.asarray(
        inputs["mp_w2"], np.float32)
    common = {
        "xt": xt8,
        "lin_w0": w("lin_w0", NP_FP8), "lin_w1": w("lin_w1"),
        "agg_w0": w("agg_w0"), "agg_w1": w("agg_w1"),
        "mp_w12": np.ascontiguousarray(w12).astype(NP_BF16),
    }
    in_maps = []
    for c in range(N_CORES):
        lo, hi = c * SHARD, (c + 1) * SHARD
        in_maps.append({
            **common,
            "xt_sh": np.ascontiguousarray(xt16[:, c * SLOTS:(c + 1) * SLOTS]),
            "a8": np.ascontiguousarray(A8[:, :, lo:hi]),
        })
    return in_maps


def run(inputs, trace=False, **kwargs):
    nc = _get_nc()
    in_maps = make_in_maps(inputs)
    res = run_bass_kernel_spmd(nc, in_maps, core_ids=list(range(N_CORES)),
                               trace=trace, **kwargs)
    out = np.concatenate([res.results[c]["out"] for c in range(N_CORES)],
                         axis=0)
    return out.astype(np.float32), res


def kernel(**inputs):
    out, _ = run(inputs, trace=False)
    return out
